# revision 1
# baseline (speedup 1.0000x reference)
"""Trainium2 Bass kernel for nn_DecoderMinLSTMGNN (v3, optimized).

Model (per sample): two MinLSTM layers (D=512) over T=4096 steps, residual,
LayerNorm, projection D->1.  B=8 samples are data-parallel across the 8
NeuronCores (one sample per core).

Measured-rate design (this part's PE tops out at ~1.35 GHz; GpSimd
elementwise is ~4 us/op and poisons the shared DVE port; bf16 DVE ops are
2x SLOWER than fp32; custom-DVE/softplus/divide unsupported by walrus):

 - ALL gate matmuls (f, i, h~; both layers) in fp8 e4m3 DoubleRow mode:
   K=256 contracted per 379 ns instruction -> 2x over bf16/fp32r.
   Layer-1 rhs is host-prepped fp8 x'; layer-2 rhs is the layer-1 scan
   output written directly as fp8 pairs (layout already matches DoubleRow).
 - h-gate biases folded away (bias-shift trick): scan runs in g = h - beta
   space with init -beta; beta2 solves (I + Wh1 Wh0) beta2 = bh1 + Wh1 bh0,
   beta1 = bh0 - Wh0 beta2; x' = x + beta2; f/i biases absorb W beta.
 - ScalarE: sigmoids (per-group bias) + reciprocal on [128,2048] quads,
   emitted in sigmoid-phase / reciprocal-phase order per outer step so the
   act-table loads stay ~2 per step instead of 97 total.
 - DVE (all fp32): den = f+i and a = f*r on [128,2048] quads,
   u2 = (a-1)*zh on [128,1024] pairs (PSUM read), time scans per group,
   residual adds on pairs.
 - LN/output stats accumulate in one packed PSUM bank (s13 rows 0..39,
   s2 rows 64..71) via matmuls against [ones | W_out*ln_g].
 - DMA ordered so t=0 tiles + weights land first.
"""

import numpy as np
import ml_dtypes

import concourse.bass as bass
import concourse.mybir as mybir
import concourse.tile as tile
from concourse.bass_utils import run_bass_kernel_spmd

F32 = mybir.dt.float32
F32R = mybir.dt.float32r
BF16 = mybir.dt.bfloat16
FP8 = mybir.dt.float8e4
AF = mybir.ActivationFunctionType
OP = mybir.AluOpType
DR = mybir.MatmulPerfMode.DoubleRow

B, T, D = 8, 4096, 512
OUT = 1
LN_EPS = 1e-5
TT = 512                 # time-tile size
NT = T // TT             # 8 time tiles
G = D // 128             # 4 channel groups
K = D // 128             # 4 contraction chunks
KP = K // 2              # 2 contraction pairs (fp8 DoubleRow)
GP = G // 2              # 2 group pairs

MAX_WAITS = 1


def _split_excess_waits(nc):
    """walrus in this container rejects >1 semaphore wait per instruction
    ("Too many sync wait commands"); move excess waits onto NoOps."""
    for fn in nc.m.functions:
        for bb in fn.blocks:
            new_list = []
            changed = False
            for inst in bb.instructions:
                si = inst.sync_info
                waits = list(si.on_wait) if si is not None and si.on_wait else []
                if len(waits) > MAX_WAITS:
                    changed = True
                    overflow = waits[:-MAX_WAITS]
                    si.on_wait = waits[-MAX_WAITS:]
                    for j in range(0, len(overflow), MAX_WAITS):
                        new_list.append(mybir.InstNoOp(
                            name=f"{inst.name}-waitsplit-{j}",
                            engine=inst.engine,
                            ins=[], outs=[],
                            sync_info=mybir.SyncInfo(
                                on_wait=overflow[j:j + MAX_WAITS], on_update=[]),
                        ))
                new_list.append(inst)
            if changed:
                bb.instructions[:] = new_list
    return nc


def _act_direct(nc, out, in_, func, bias=0.0, scale=1.0):
    """emit InstActivation directly (bass blocks Reciprocal/Rsqrt)."""
    ins = [nc.scalar.lower_ap(in_)]
    for v in (bias, scale, 0.0):
        if isinstance(v, (int, float)):
            ins.append(mybir.ImmediateValue(dtype=mybir.dt.float32, value=float(v)))
        else:
            ins.append(nc.scalar.lower_ap(v))
    return nc.scalar.add_instruction(
        mybir.InstActivation(
            name=nc.get_next_instruction_name(),
            func=func, ins=ins, outs=[nc.scalar.lower_ap(out)]))


def _build_nc(split_waits=True):
    nc = bass.Bass()

    # fp8 interleaved x' for all layer-1 gates: [kp, p, j, T]
    x8_d = nc.dram_tensor("x8", [KP, 128, 2, T], FP8, kind="ExternalInput")
    # bf16 x' for the residual: [kp, p, j, T]
    xtb_d = nc.dram_tensor("xtb", [KP, 128, 2, T], BF16, kind="ExternalInput")
    # fp8 weights, both layers: [layer, gate(f,i,h), kp, p, j, m]
    w8_d = nc.dram_tensor("w8", [2, 3, KP, 128, 2, D], FP8, kind="ExternalInput")
    # f/i sigmoid biases: bias[p, layer, {f,i}, g] = b'[g*128+p]
    bias_d = nc.dram_tensor("bias", [128, 2, 2, G], F32, kind="ExternalInput")
    # scan inits: binit[p, layer, g] = -beta_layer[g*128+p]
    binit_d = nc.dram_tensor("binit", [128, 2, G], F32, kind="ExternalInput")
    # stats lhsT per (g,t): col t = 1, col 32+t = wg[g*128:(g+1)*128]
    slt_d = nc.dram_tensor("slt", [G, NT, 128, 40], F32R, kind="ExternalInput")
    # S2 lhsT per t: col t = 1  (bf16: rhs sq is bf16)
    s2l_d = nc.dram_tensor("s2l", [NT, 128, NT], BF16, kind="ExternalInput")
    epi_d = nc.dram_tensor("epi", [NT, 3], F32, kind="ExternalInput")  # [c0, swg/D, eps]
    out_d = nc.dram_tensor("out", [NT, TT], F32, kind="ExternalOutput")

    with tile.TileContext(nc) as tc:
        with (
            tc.tile_pool(name="const", bufs=1) as const,
            tc.tile_pool(name="xp", bufs=1) as xp,
            tc.tile_pool(name="hp", bufs=1) as hp,
            tc.tile_pool(name="fi", bufs=2) as fi,
            tc.tile_pool(name="work", bufs=2) as work,
            tc.tile_pool(name="g2p", bufs=2) as g2p,
            tc.tile_pool(name="ep", bufs=2) as ep,
            tc.tile_pool(name="fin", bufs=1) as fin,
            tc.tile_pool(name="pfi_ps", bufs=3, space="PSUM") as pfi_ps,
            tc.tile_pool(name="ph_ps", bufs=1, space="PSUM") as ph_ps,
            tc.tile_pool(name="stats_ps", bufs=1, space="PSUM") as stats_ps,
        ):
            # ---- weights / constants, ordered for fast pipeline start ----
            w8_sb = const.tile([128, 2, 3, KP, 2, D], FP8, tag="w8")
            nc.sync.dma_start(
                out=w8_sb[:],
                in_=w8_d.rearrange("l gate kp p j m -> p l gate kp j m"))
            # fp8 x: one DMA per kp covering all T
            x8_sb = []
            for kp in range(KP):
                x8t = const.tile([128, 2, T], FP8, tag=f"x8_{kp}")
                nc.sync.dma_start(out=x8t[:], in_=x8_d[kp])
                x8_sb.append(x8t)
            bias_sb = const.tile([128, 2, 2, G], F32)
            nc.sync.dma_start(out=bias_sb[:], in_=bias_d[:])
            binit_sb = const.tile([128, 2, G], F32)
            nc.sync.dma_start(out=binit_sb[:], in_=binit_d[:])

            # bf16 x tiles per (kp, tpair): [128, 2, 2*TT] (residual only;
            # rotating pool - consumed by the epilogue in tp order)
            xtb_sb = [[None] * (NT // 2) for _ in range(KP)]
            for tp in range(NT // 2):
                for kp in range(KP):
                    xx = xp.tile([128, 2, 2 * TT], BF16, tag=f"xtb{kp}",
                                 name=f"xtb{kp}_{tp}", bufs=2)
                    nc.sync.dma_start(
                        out=xx[:], in_=xtb_d[kp, :, :, tp * 2 * TT:(tp + 1) * 2 * TT])
                    xtb_sb[kp][tp] = xx
                if tp == 0:
                    slt_sb = const.tile([128, G, NT, 40], F32R)
                    nc.sync.dma_start(
                        out=slt_sb[:], in_=slt_d.rearrange("g t p c -> p g t c"))
                    s2l_sb = const.tile([128, NT, NT], BF16)
                    nc.sync.dma_start(out=s2l_sb[:], in_=s2l_d.rearrange("t p c -> p t c"))
                    epi_sb = const.tile([NT, 3], F32)
                    nc.sync.dma_start(out=epi_sb[:], in_=epi_d[:])

            # stats accumulate in ONE packed PSUM bank:
            #   rows 0..39  = s13 (s1 in cols t, s3 in cols 32+t)
            #   rows 64..71 = s2
            st_ps = stats_ps.tile([128, TT], F32, tag="st")
            s13_ps = st_ps[0:40, :]
            s2_ps = st_ps[64:64 + NT, :]
            stats_first = [True]

            # layer-1 outputs, fp8 pairs, per (gpair, t): [128, 2, TT]
            h1_sb = [[None] * NT for _ in range(GP)]
            g2_sb = [[None] * NT for _ in range(GP)]  # layer-2 scan outputs (F32)

            def gate_mms_and_sig(layer, t):
                """gate matmuls + sigmoid emissions for one (layer, t) block"""
                rhs_pair = (x8_sb if layer == 0 else
                            [h1_sb[kp][t] for kp in range(KP)])
                f_quad = fi.tile([128, G * TT], F32, tag="f")
                i_quad = fi.tile([128, G * TT], F32, tag="i")
                ph_l = []
                for gp in range(GP):
                    ph = ph_ps.tile([128, 2 * TT], F32, tag=f"ph{gp}")
                    ph_l.append(ph)
                for g in range(G):
                    gp, gj = g // 2, g % 2
                    for gate, dst in ((0, None), (1, None), (2, ph_l[gp])):
                        if gate < 2:
                            ps = pfi_ps.tile([128, TT], F32, tag="pfi")
                        for kp in range(KP):
                            rhs = rhs_pair[kp]
                            rr = (rhs[:, :, t * TT:(t + 1) * TT]
                                  if layer == 0 else rhs[:])
                            nc.tensor.matmul(
                                ps[:] if gate < 2 else dst[:, gj * TT:(gj + 1) * TT],
                                w8_sb[:, layer, gate, kp, :, g * 128:(g + 1) * 128],
                                rr, start=(kp == 0), stop=(kp == KP - 1),
                                perf_mode=DR)
                        if gate < 2:
                            tgt = f_quad if gate == 0 else i_quad
                            nc.scalar.activation(
                                tgt[:, g * TT:(g + 1) * TT], ps[:], AF.Sigmoid,
                                bias=bias_sb[:, layer, gate, g:g + 1])
                return f_quad, i_quad, ph_l

            def den_phase(blk):
                f_quad, i_quad, ph_l, layer, t = blk
                den = work.tile([128, G * TT], F32, tag="den")
                nc.vector.tensor_add(den[:], f_quad[:], i_quad[:])
                return den

            def recip_phase(blk, den):
                r = work.tile([128, G * TT], F32, tag="r")
                _act_direct(nc, r[:], den[:], AF.Reciprocal)
                return r

            def scan_phase(blk, r):
                f_quad, i_quad, ph_l, layer, t = blk
                a_quad = work.tile([128, G * TT], F32, tag="a")
                nc.vector.tensor_mul(a_quad[:], f_quad[:], r[:])
                if layer == 0:
                    h_pairs = [hp.tile([128, 2, TT], FP8, tag=f"h1_{gp}_{t}",
                                       name=f"h1_{gp}_{t}")
                               for gp in range(GP)]
                    for gp in range(GP):
                        h1_sb[gp][t] = h_pairs[gp]
                    prev = ([h1_sb[gp][t - 1] for gp in range(GP)]
                            if t > 0 else None)
                else:
                    h_pairs = [g2p.tile([128, 2, TT], F32, tag=f"g2_{gp}",
                                        name=f"g2_{gp}_{t}")
                               for gp in range(GP)]
                    for gp in range(GP):
                        g2_sb[gp][t] = h_pairs[gp]
                    prev = ([g2_sb[gp][t - 1] for gp in range(GP)]
                            if t > 0 else None)
                for gp in range(GP):
                    u2 = work.tile([128, 2 * TT], F32, tag="u2")
                    nc.vector.scalar_tensor_tensor(
                        u2[:], a_quad[:, gp * 2 * TT:(gp + 1) * 2 * TT], 1.0,
                        ph_l[gp][:], OP.subtract, OP.mult)
                    for gj in range(2):
                        g = gp * 2 + gj
                        if t == 0:
                            init = binit_sb[:, layer, g:g + 1]
                        else:
                            init = prev[gp][:, gj, TT - 1:TT]
                        nc.vector.tensor_tensor_scan(
                            h_pairs[gp][:, gj, :],
                            a_quad[:, g * TT:(g + 1) * TT],
                            u2[:, gj * TT:(gj + 1) * TT],
                            init, OP.mult, OP.subtract)

            def epilogue_tile(t):
                """residual + LN/output stats for one time tile"""
                for gp in range(GP):
                    res = ep.tile([128, 2, TT], F32R, tag="res")
                    nc.gpsimd.tensor_tensor(
                        res[:], g2_sb[gp][t][:],
                        xtb_sb[gp][t // 2][:, :, (t % 2) * TT:(t % 2 + 1) * TT],
                        OP.add)
                    sq = ep.tile([128, 2, TT], BF16, tag="sq")
                    nc.scalar.activation(sq[:], res[:].bitcast(F32), AF.Square)
                    first = stats_first[0]
                    stats_first[0] = False
                    last = (t == NT - 1 and gp == GP - 1)
                    for gj in range(2):
                        g = gp * 2 + gj
                        nc.tensor.matmul(
                            s13_ps, slt_sb[:, g, t, :],
                            res[:, gj, :],
                            start=first and gj == 0,
                            stop=last and gj == 1, skip_group_check=True)
                        nc.tensor.matmul(
                            s2_ps, s2l_sb[:, t, :], sq[:, gj, :],
                            start=first and gj == 0,
                            stop=last and gj == 1, skip_group_check=True)

            # ---- pipeline: per outer step emit both blocks' matmuls+sigmoids,
            # then their dens, then both reciprocals (table phase), then scans.
            def outer(t):
                blks = []
                blk0 = (*gate_mms_and_sig(0, t), 0, t)
                blks.append(blk0)
                if t >= 1:
                    blk1 = (*gate_mms_and_sig(1, t - 1), 1, t - 1)
                    blks.append(blk1)
                dens = [den_phase(b) for b in blks]
                rs = [recip_phase(b, d) for b, d in zip(blks, dens)]
                for b, r in zip(blks, rs):
                    scan_phase(b, r)
                if t >= 2:
                    epilogue_tile(t - 2)

            for t in range(NT):
                outer(t)
            # drain: layer-2 final tile, last epilogues
            blk1 = (*gate_mms_and_sig(1, NT - 1), 1, NT - 1)
            den = den_phase(blk1)
            r = recip_phase(blk1, den)
            scan_phase(blk1, r)
            epilogue_tile(NT - 2)
            epilogue_tile(NT - 1)

            # ---- final LN + projection math on [8, 512] ----
            s1 = st_ps[0:NT, :]
            s3p = st_ps[32:32 + NT, :]
            s3_sb = fin.tile([NT, TT], F32, tag="s3f")
            nc.scalar.activation(s3_sb[:], s3p, AF.Copy)
            # nn = (s1 * swg/D) - s3
            nn_sb = fin.tile([NT, TT], F32, tag="nn")
            nc.vector.scalar_tensor_tensor(
                nn_sb[:], s1, epi_sb[:, 1:2], s3_sb[:], OP.mult, OP.subtract)
            # s1sq = (s1/D)^2
            s1sq_sb = fin.tile([NT, TT], F32, tag="s1sq")
            nc.scalar.activation(s1sq_sb[:], s1, AF.Square, scale=1.0 / D)
            # v = s2/D - s1sq
            v_sb = fin.tile([NT, TT], F32, tag="v")
            nc.vector.scalar_tensor_tensor(
                v_sb[:], s2_ps, 1.0 / D, s1sq_sb[:], OP.mult, OP.subtract)
            # rv = rsqrt(v + eps)  (one act-table switch, at the very end)
            rv_sb = fin.tile([NT, TT], F32, tag="rv")
            _act_direct(nc, rv_sb[:], v_sb[:], AF.Rsqrt, bias=epi_sb[:, 2:3])
            # pr = (nn * -1) * rv = (s3 - mu*swg) * rv
            pr_sb = fin.tile([NT, TT], F32, tag="pr")
            nc.vector.scalar_tensor_tensor(
                pr_sb[:], nn_sb[:], -1.0, rv_sb[:], OP.mult, OP.mult)
            # out = pr + c0
            o_sb = fin.tile([NT, TT], F32, tag="o")
            nc.scalar.activation(o_sb[:], pr_sb[:], AF.Identity,
                                 bias=epi_sb[:, 0:1])
            nc.sync.dma_start(out=out_d[:], in_=o_sb[:])

    if split_waits:
        _split_excess_waits(nc)
    return nc


_NC_CACHE = None


def _get_nc():
    global _NC_CACHE
    if _NC_CACHE is None:
        _NC_CACHE = _build_nc()
    return _NC_CACHE


def _host_prep(inputs):
    x = np.asarray(inputs["x"], dtype=np.float64)
    W = {k: np.asarray(inputs[k], np.float64)
         for k in ("Wf0", "Wi0", "Wh0", "Wf1", "Wi1", "Wh1")}
    b = {k: np.asarray(inputs[k], np.float64)
         for k in ("bf0", "bi0", "bh0", "bf1", "bi1", "bh1")}

    # bias folding: h = g + beta per layer; res = x + g2 + beta2 = x' + g2
    beta2 = np.linalg.solve(
        np.eye(D) + W["Wh1"] @ W["Wh0"], b["bh1"] + W["Wh1"] @ b["bh0"])
    beta1 = b["bh0"] - W["Wh0"] @ beta2
    xp = (x + beta2).astype(np.float32)          # [B, T, D]
    sig_bias = {
        (0, 0): b["bf0"] - W["Wf0"] @ beta2,
        (0, 1): b["bi0"] - W["Wi0"] @ beta2,
        (1, 0): b["bf1"] + W["Wf1"] @ beta1,
        (1, 1): b["bi1"] + W["Wi1"] @ beta1,
    }
    bias_all = np.zeros((128, 2, 2, G), np.float32)
    for layer in range(2):
        for j in range(2):
            bias_all[:, layer, j, :] = (
                sig_bias[(layer, j)].astype(np.float32).reshape(G, 128).T)
    binit = np.zeros((128, 2, G), np.float32)
    binit[:, 0, :] = (-beta1).astype(np.float32).reshape(G, 128).T
    binit[:, 1, :] = (-beta2).astype(np.float32).reshape(G, 128).T

    # fp8 weights, both layers: w8[l, gate, kp, p, j, m] = W[m, (2kp+j)*128+p]
    w8 = np.zeros((2, 3, KP, 128, 2, D), np.float32)
    for li, names in enumerate((("Wf0", "Wi0", "Wh0"), ("Wf1", "Wi1", "Wh1"))):
        for gi, wk in enumerate(names):
            wm = W[wk].astype(np.float32)        # [m, k]
            for kp in range(KP):
                for j in range(2):
                    w8[li, gi, kp, :, j, :] = \
                        wm[:, (2 * kp + j) * 128:(2 * kp + j + 1) * 128].T
    w8 = np.clip(w8, -240, 240).astype(ml_dtypes.float8_e4m3fn)

    # epilogue constants
    w_out = np.asarray(inputs["W_out"], np.float32).reshape(D)
    ln_g = np.asarray(inputs["ln_g"], np.float32)
    ln_b = np.asarray(inputs["ln_b"], np.float32)
    b_out = np.asarray(inputs["b_out"], np.float32).reshape(())
    wg = w_out * ln_g
    c0 = float(np.dot(w_out, ln_b) + b_out)
    swg = float(wg.sum())
    slt = np.zeros((G, NT, 128, 40), np.float32)
    for g in range(G):
        for t in range(NT):
            slt[g, t, :, t] = 1.0
            slt[g, t, :, 32 + t] = wg[g * 128:(g + 1) * 128]
    s2l = np.zeros((NT, 128, NT), np.float32)
    for t in range(NT):
        s2l[t, :, t] = 1.0
    s2l = s2l.astype(ml_dtypes.bfloat16)
    epi = np.zeros((NT, 3), np.float32)
    epi[:, 0] = c0
    epi[:, 1] = swg / D
    epi[:, 2] = LN_EPS
    return xp, w8, bias_all, binit, slt, s2l, epi


def _in_maps(inputs):
    xp, w8, bias_all, binit, slt, s2l, epi = _host_prep(inputs)
    maps = []
    for bi in range(B):
        xt = xp[bi].T                            # [D, T] fp32
        # [kp, p, j, T] views
        xk = np.ascontiguousarray(
            xt.reshape(KP, 2, 128, T).transpose(0, 2, 1, 3))
        x8 = np.clip(xk, -240, 240).astype(ml_dtypes.float8_e4m3fn)
        xtb = xk.astype(ml_dtypes.bfloat16)
        maps.append({
            "x8": x8, "xtb": xtb, "w8": w8,
            "bias": bias_all, "binit": binit,
            "slt": slt, "s2l": s2l, "epi": epi,
        })
    return maps


def kernel(**inputs):
    nc = _get_nc()
    res = run_bass_kernel_spmd(nc, _in_maps(inputs), list(range(B)))
    out = np.stack([res.results[b]["out"].reshape(T, OUT) for b in range(B)])
    return out.astype(np.float32)


def kernel_traced(**inputs):
    """same as kernel() but returns (output, BassKernelResults) with timing"""
    nc = _get_nc()
    res = run_bass_kernel_spmd(nc, _in_maps(inputs), list(range(B)), trace=True)
    out = np.stack([res.results[b]["out"].reshape(T, OUT) for b in range(B)])
    return out.astype(np.float32), res



# revision 10
# speedup vs baseline: 1.2467x; 1.2467x over previous
"""Trainium2 Bass kernel for nn_DecoderMinLSTMGNN (v3, optimized).

Model (per sample): two MinLSTM layers (D=512) over T=4096 steps, residual,
LayerNorm, projection D->1.  B=8 samples are data-parallel across the 8
NeuronCores (one sample per core).

Measured-rate design (this part's PE tops out at ~1.35 GHz; GpSimd
elementwise is ~4 us/op and poisons the shared DVE port; bf16 DVE ops are
2x SLOWER than fp32; custom-DVE/softplus/divide unsupported by walrus):

 - ALL gate matmuls (f, i, h~; both layers) in fp8 e4m3 DoubleRow mode:
   K=256 contracted per 379 ns instruction -> 2x over bf16/fp32r.
   Layer-1 rhs is host-prepped fp8 x'; layer-2 rhs is the layer-1 scan
   output written directly as fp8 pairs (layout already matches DoubleRow).
 - h-gate biases folded away (bias-shift trick): scan runs in g = h - beta
   space with init -beta; beta2 solves (I + Wh1 Wh0) beta2 = bh1 + Wh1 bh0,
   beta1 = bh0 - Wh0 beta2; x' = x + beta2; f/i biases absorb W beta.
 - ScalarE: sigmoids (per-group bias) + reciprocal on [128,2048] quads,
   emitted in sigmoid-phase / reciprocal-phase order per outer step so the
   act-table loads stay ~2 per step instead of 97 total.
 - DVE (all fp32): den = f+i and a = f*r on [128,2048] quads,
   u2 = (a-1)*zh on [128,1024] pairs (PSUM read), time scans per group,
   residual adds on pairs.
 - LN/output stats accumulate in one packed PSUM bank (s13 rows 0..39,
   s2 rows 64..71) via matmuls against [ones | W_out*ln_g].
 - DMA ordered so t=0 tiles + weights land first.
"""

import numpy as np
import ml_dtypes

import concourse.bass as bass
import concourse.mybir as mybir
import concourse.tile as tile
from concourse.bass_utils import run_bass_kernel_spmd

F32 = mybir.dt.float32
F32R = mybir.dt.float32r
BF16 = mybir.dt.bfloat16
FP8 = mybir.dt.float8e4
AF = mybir.ActivationFunctionType
OP = mybir.AluOpType
DR = mybir.MatmulPerfMode.DoubleRow

B, T, D = 8, 4096, 512
OUT = 1
LN_EPS = 1e-5
TT = 512                 # time-tile size
NT = T // TT             # 8 time tiles
G = D // 128             # 4 channel groups
K = D // 128             # 4 contraction chunks
KP = K // 2              # 2 contraction pairs (fp8 DoubleRow)
GP = G // 2              # 2 group pairs

MAX_WAITS = 1


def _split_excess_waits(nc):
    """walrus in this container rejects >1 semaphore wait per instruction
    ("Too many sync wait commands"); move excess waits onto NoOps."""
    for fn in nc.m.functions:
        for bb in fn.blocks:
            new_list = []
            changed = False
            for inst in bb.instructions:
                si = inst.sync_info
                waits = list(si.on_wait) if si is not None and si.on_wait else []
                if len(waits) > MAX_WAITS:
                    changed = True
                    overflow = waits[:-MAX_WAITS]
                    si.on_wait = waits[-MAX_WAITS:]
                    for j in range(0, len(overflow), MAX_WAITS):
                        new_list.append(mybir.InstNoOp(
                            name=f"{inst.name}-waitsplit-{j}",
                            engine=inst.engine,
                            ins=[], outs=[],
                            sync_info=mybir.SyncInfo(
                                on_wait=overflow[j:j + MAX_WAITS], on_update=[]),
                        ))
                new_list.append(inst)
            if changed:
                bb.instructions[:] = new_list
    return nc


def _act_direct(nc, out, in_, func, bias=0.0, scale=1.0):
    """emit InstActivation directly (bass blocks Reciprocal/Rsqrt)."""
    ins = [nc.scalar.lower_ap(in_)]
    for v in (bias, scale, 0.0):
        if isinstance(v, (int, float)):
            ins.append(mybir.ImmediateValue(dtype=mybir.dt.float32, value=float(v)))
        else:
            ins.append(nc.scalar.lower_ap(v))
    return nc.scalar.add_instruction(
        mybir.InstActivation(
            name=nc.get_next_instruction_name(),
            func=func, ins=ins, outs=[nc.scalar.lower_ap(out)]))


def _build_nc(split_waits=True):
    nc = bass.Bass()

    # fp8 interleaved x' for all layer-1 gates: [kp, p, j, T]
    x8_d = nc.dram_tensor("x8", [KP, 128, 2, T], FP8, kind="ExternalInput")
    # bf16 x' for the residual: [kp, p, j, T]
    xtb_d = nc.dram_tensor("xtb", [KP, 128, 2, T], BF16, kind="ExternalInput")
    # fp8 weights, both layers: [layer, gate(f,i,h), kp, p, j, m]
    w8_d = nc.dram_tensor("w8", [2, 3, KP, 128, 2, D], FP8, kind="ExternalInput")
    # f/i sigmoid biases: bias[p, layer, {f,i}, g] = b'[g*128+p]
    bias_d = nc.dram_tensor("bias", [128, 2, 2, G], F32, kind="ExternalInput")
    # scan inits: binit[p, layer, g] = -beta_layer[g*128+p]
    binit_d = nc.dram_tensor("binit", [128, 2, G], F32, kind="ExternalInput")
    # stats lhsT per (g,t): col t = 1, col 32+t = wg[g*128:(g+1)*128]
    slt_d = nc.dram_tensor("slt", [G, NT, 128, 40], BF16, kind="ExternalInput")
    # S2 lhsT per t: col t = 1  (bf16: rhs sq is bf16)
    s2l_d = nc.dram_tensor("s2l", [NT, 128, NT], BF16, kind="ExternalInput")
    epi_d = nc.dram_tensor("epi", [NT, 3], F32, kind="ExternalInput")  # [c0, swg/D, eps]
    out_d = nc.dram_tensor("out", [NT, TT], F32, kind="ExternalOutput")

    with tile.TileContext(nc) as tc:
        with (
            tc.tile_pool(name="const", bufs=1) as const,
            tc.tile_pool(name="xp", bufs=1) as xp,
            tc.tile_pool(name="hp", bufs=1) as hp,
            tc.tile_pool(name="fi", bufs=2) as fi,
            tc.tile_pool(name="work", bufs=2) as work,
            tc.tile_pool(name="g2p", bufs=2) as g2p,
            tc.tile_pool(name="ep", bufs=2) as ep,
            tc.tile_pool(name="fin", bufs=1) as fin,
            tc.tile_pool(name="pfi_ps", bufs=3, space="PSUM") as pfi_ps,
            tc.tile_pool(name="ph_ps", bufs=1, space="PSUM") as ph_ps,
            tc.tile_pool(name="stats_ps", bufs=1, space="PSUM") as stats_ps,
        ):
            # ---- weights / constants, ordered for fast pipeline start ----
            w8_sb = const.tile([128, 2, 3, KP, 2, D], FP8, tag="w8")
            nc.sync.dma_start(
                out=w8_sb[:],
                in_=w8_d.rearrange("l gate kp p j m -> p l gate kp j m"))
            # fp8 x: one DMA per kp covering all T
            x8_sb = []
            for kp in range(KP):
                x8t = const.tile([128, 2, T], FP8, tag=f"x8_{kp}")
                nc.sync.dma_start(out=x8t[:], in_=x8_d[kp])
                x8_sb.append(x8t)
            bias_sb = const.tile([128, 2, 2, G], F32)
            nc.sync.dma_start(out=bias_sb[:], in_=bias_d[:])
            binit_sb = const.tile([128, 2, G], F32)
            nc.sync.dma_start(out=binit_sb[:], in_=binit_d[:])

            # bf16 x tiles per (kp, tpair): [128, 2, 2*TT] (residual only;
            # rotating pool - consumed by the epilogue in tp order)
            xtb_sb = [[None] * (NT // 2) for _ in range(KP)]
            for tp in range(NT // 2):
                for kp in range(KP):
                    xx = xp.tile([128, 2, 2 * TT], BF16, tag=f"xtb{kp}",
                                 name=f"xtb{kp}_{tp}", bufs=2)
                    nc.sync.dma_start(
                        out=xx[:], in_=xtb_d[kp, :, :, tp * 2 * TT:(tp + 1) * 2 * TT])
                    xtb_sb[kp][tp] = xx
                if tp == 0:
                    slt_sb = const.tile([128, G, NT, 40], BF16)
                    nc.sync.dma_start(
                        out=slt_sb[:], in_=slt_d.rearrange("g t p c -> p g t c"))
                    s2l_sb = const.tile([128, NT, NT], BF16)
                    nc.sync.dma_start(out=s2l_sb[:], in_=s2l_d.rearrange("t p c -> p t c"))
                    epi_sb = const.tile([NT, 3], F32)
                    nc.sync.dma_start(out=epi_sb[:], in_=epi_d[:])

            # stats accumulate in ONE packed PSUM bank:
            #   rows 0..39  = s13 (s1 in cols t, s3 in cols 32+t)
            #   rows 64..71 = s2
            st_ps = stats_ps.tile([128, TT], F32, tag="st")
            s13_ps = st_ps[0:40, :]
            s2_ps = st_ps[64:64 + NT, :]
            stats_first = [True]

            # layer-1 outputs, fp8 pairs, per (gpair, t): [128, 2, TT]
            h1_sb = [[None] * NT for _ in range(GP)]
            g2_sb = [[None] * NT for _ in range(GP)]  # layer-2 scan outputs (F32)

            def gate_mms_and_sig(layer, t):
                """gate matmuls + sigmoid emissions for one (layer, t) block"""
                rhs_pair = (x8_sb if layer == 0 else
                            [h1_sb[kp][t] for kp in range(KP)])
                f_quad = fi.tile([128, G * TT], BF16, tag="f")
                i_quad = fi.tile([128, G * TT], BF16, tag="i")
                ph_l = []
                for gp in range(GP):
                    ph = ph_ps.tile([128, 2 * TT], F32, tag=f"ph{gp}")
                    ph_l.append(ph)
                for g in range(G):
                    gp, gj = g // 2, g % 2
                    for gate, dst in ((0, None), (1, None), (2, ph_l[gp])):
                        if gate < 2:
                            ps = pfi_ps.tile([128, TT], F32, tag="pfi")
                        for kp in range(KP):
                            rhs = rhs_pair[kp]
                            rr = (rhs[:, :, t * TT:(t + 1) * TT]
                                  if layer == 0 else rhs[:])
                            nc.tensor.matmul(
                                ps[:] if gate < 2 else dst[:, gj * TT:(gj + 1) * TT],
                                w8_sb[:, layer, gate, kp, :, g * 128:(g + 1) * 128],
                                rr, start=(kp == 0), stop=(kp == KP - 1),
                                perf_mode=DR)
                        if gate < 2:
                            tgt = f_quad if gate == 0 else i_quad
                            nc.scalar.activation(
                                tgt[:, g * TT:(g + 1) * TT], ps[:], AF.Sigmoid,
                                bias=bias_sb[:, layer, gate, g:g + 1])
                return f_quad, i_quad, ph_l

            def den_phase(blk):
                f_quad, i_quad, ph_l, layer, t = blk
                den = work.tile([128, G * TT], BF16, tag="den")
                nc.vector.tensor_add(den[:], f_quad[:], i_quad[:])
                return den

            def recip_phase(blk, den):
                r = work.tile([128, G * TT], BF16, tag="r")
                _act_direct(nc, r[:], den[:], AF.Reciprocal)
                return r

            def scan_phase(blk, r):
                f_quad, i_quad, ph_l, layer, t = blk
                a_quad = work.tile([128, G * TT], BF16, tag="a")
                nc.vector.tensor_mul(a_quad[:], f_quad[:], r[:])
                if layer == 0:
                    h_pairs = [hp.tile([128, 2, TT], FP8, tag=f"h1_{gp}_{t}",
                                       name=f"h1_{gp}_{t}")
                               for gp in range(GP)]
                    for gp in range(GP):
                        h1_sb[gp][t] = h_pairs[gp]
                    prev = ([h1_sb[gp][t - 1] for gp in range(GP)]
                            if t > 0 else None)
                else:
                    h_pairs = [g2p.tile([128, 2, TT], BF16, tag=f"g2_{gp}",
                                        name=f"g2_{gp}_{t}")
                               for gp in range(GP)]
                    for gp in range(GP):
                        g2_sb[gp][t] = h_pairs[gp]
                    prev = ([g2_sb[gp][t - 1] for gp in range(GP)]
                            if t > 0 else None)
                for gp in range(GP):
                    u2 = work.tile([128, 2 * TT], BF16, tag="u2")
                    nc.vector.scalar_tensor_tensor(
                        u2[:], a_quad[:, gp * 2 * TT:(gp + 1) * 2 * TT], 1.0,
                        ph_l[gp][:], OP.subtract, OP.mult)
                    for gj in range(2):
                        g = gp * 2 + gj
                        if t == 0:
                            init = binit_sb[:, layer, g:g + 1]
                        else:
                            init = prev[gp][:, gj, TT - 1:TT]
                        nc.vector.tensor_tensor_scan(
                            h_pairs[gp][:, gj, :],
                            a_quad[:, g * TT:(g + 1) * TT],
                            u2[:, gj * TT:(gj + 1) * TT],
                            init, OP.mult, OP.subtract)

            def epilogue_tile(t):
                """residual + LN/output stats for one time tile"""
                for gp in range(GP):
                    res = ep.tile([128, 2, TT], BF16, tag="res")
                    nc.vector.tensor_add(
                        res[:], g2_sb[gp][t][:],
                        xtb_sb[gp][t // 2][:, :, (t % 2) * TT:(t % 2 + 1) * TT])
                    sq = ep.tile([128, 2, TT], BF16, tag="sq")
                    nc.scalar.activation(sq[:], res[:], AF.Square)
                    first = stats_first[0]
                    stats_first[0] = False
                    last = (t == NT - 1 and gp == GP - 1)
                    for gj in range(2):
                        g = gp * 2 + gj
                        nc.tensor.matmul(
                            s13_ps, slt_sb[:, g, t, :],
                            res[:, gj, :],
                            start=first and gj == 0,
                            stop=last and gj == 1, skip_group_check=True)
                        nc.tensor.matmul(
                            s2_ps, s2l_sb[:, t, :], sq[:, gj, :],
                            start=first and gj == 0,
                            stop=last and gj == 1, skip_group_check=True)

            # ---- pipeline: per outer step emit both blocks' matmuls+sigmoids,
            # then their dens, then both reciprocals (table phase), then scans.
            def outer(t):
                blks = []
                blk0 = (*gate_mms_and_sig(0, t), 0, t)
                blks.append(blk0)
                if t >= 1:
                    blk1 = (*gate_mms_and_sig(1, t - 1), 1, t - 1)
                    blks.append(blk1)
                dens = [den_phase(b) for b in blks]
                rs = [recip_phase(b, d) for b, d in zip(blks, dens)]
                for b, r in zip(blks, rs):
                    scan_phase(b, r)
                if t >= 2:
                    epilogue_tile(t - 2)

            for t in range(NT):
                outer(t)
            # drain: layer-2 final tile, last epilogues
            blk1 = (*gate_mms_and_sig(1, NT - 1), 1, NT - 1)
            den = den_phase(blk1)
            r = recip_phase(blk1, den)
            scan_phase(blk1, r)
            epilogue_tile(NT - 2)
            epilogue_tile(NT - 1)

            # ---- final LN + projection math on [8, 512] ----
            s1 = st_ps[0:NT, :]
            s3p = st_ps[32:32 + NT, :]
            s3_sb = fin.tile([NT, TT], F32, tag="s3f")
            nc.scalar.activation(s3_sb[:], s3p, AF.Copy)
            # nn = (s1 * swg/D) - s3
            nn_sb = fin.tile([NT, TT], F32, tag="nn")
            nc.vector.scalar_tensor_tensor(
                nn_sb[:], s1, epi_sb[:, 1:2], s3_sb[:], OP.mult, OP.subtract)
            # s1sq = (s1/D)^2
            s1sq_sb = fin.tile([NT, TT], F32, tag="s1sq")
            nc.scalar.activation(s1sq_sb[:], s1, AF.Square, scale=1.0 / D)
            # v = s2/D - s1sq
            v_sb = fin.tile([NT, TT], F32, tag="v")
            nc.vector.scalar_tensor_tensor(
                v_sb[:], s2_ps, 1.0 / D, s1sq_sb[:], OP.mult, OP.subtract)
            # rv = rsqrt(v + eps)  (one act-table switch, at the very end)
            rv_sb = fin.tile([NT, TT], F32, tag="rv")
            _act_direct(nc, rv_sb[:], v_sb[:], AF.Rsqrt, bias=epi_sb[:, 2:3])
            # pr = (nn * -1) * rv = (s3 - mu*swg) * rv
            pr_sb = fin.tile([NT, TT], F32, tag="pr")
            nc.vector.scalar_tensor_tensor(
                pr_sb[:], nn_sb[:], -1.0, rv_sb[:], OP.mult, OP.mult)
            # out = pr + c0
            o_sb = fin.tile([NT, TT], F32, tag="o")
            nc.scalar.activation(o_sb[:], pr_sb[:], AF.Identity,
                                 bias=epi_sb[:, 0:1])
            nc.sync.dma_start(out=out_d[:], in_=o_sb[:])

    if split_waits:
        _split_excess_waits(nc)
    return nc


_NC_CACHE = None


def _get_nc():
    global _NC_CACHE
    if _NC_CACHE is None:
        _NC_CACHE = _build_nc()
    return _NC_CACHE


def _host_prep(inputs):
    x = np.asarray(inputs["x"], dtype=np.float64)
    W = {k: np.asarray(inputs[k], np.float64)
         for k in ("Wf0", "Wi0", "Wh0", "Wf1", "Wi1", "Wh1")}
    b = {k: np.asarray(inputs[k], np.float64)
         for k in ("bf0", "bi0", "bh0", "bf1", "bi1", "bh1")}

    # bias folding: h = g + beta per layer; res = x + g2 + beta2 = x' + g2
    beta2 = np.linalg.solve(
        np.eye(D) + W["Wh1"] @ W["Wh0"], b["bh1"] + W["Wh1"] @ b["bh0"])
    beta1 = b["bh0"] - W["Wh0"] @ beta2
    xp = (x + beta2).astype(np.float32)          # [B, T, D]
    sig_bias = {
        (0, 0): b["bf0"] - W["Wf0"] @ beta2,
        (0, 1): b["bi0"] - W["Wi0"] @ beta2,
        (1, 0): b["bf1"] + W["Wf1"] @ beta1,
        (1, 1): b["bi1"] + W["Wi1"] @ beta1,
    }
    bias_all = np.zeros((128, 2, 2, G), np.float32)
    for layer in range(2):
        for j in range(2):
            bias_all[:, layer, j, :] = (
                sig_bias[(layer, j)].astype(np.float32).reshape(G, 128).T)
    binit = np.zeros((128, 2, G), np.float32)
    binit[:, 0, :] = (-beta1).astype(np.float32).reshape(G, 128).T
    binit[:, 1, :] = (-beta2).astype(np.float32).reshape(G, 128).T

    # fp8 weights, both layers: w8[l, gate, kp, p, j, m] = W[m, (2kp+j)*128+p]
    w8 = np.zeros((2, 3, KP, 128, 2, D), np.float32)
    for li, names in enumerate((("Wf0", "Wi0", "Wh0"), ("Wf1", "Wi1", "Wh1"))):
        for gi, wk in enumerate(names):
            wm = W[wk].astype(np.float32)        # [m, k]
            for kp in range(KP):
                for j in range(2):
                    w8[li, gi, kp, :, j, :] = \
                        wm[:, (2 * kp + j) * 128:(2 * kp + j + 1) * 128].T
    w8 = np.clip(w8, -240, 240).astype(ml_dtypes.float8_e4m3fn)

    # epilogue constants
    w_out = np.asarray(inputs["W_out"], np.float32).reshape(D)
    ln_g = np.asarray(inputs["ln_g"], np.float32)
    ln_b = np.asarray(inputs["ln_b"], np.float32)
    b_out = np.asarray(inputs["b_out"], np.float32).reshape(())
    wg = w_out * ln_g
    c0 = float(np.dot(w_out, ln_b) + b_out)
    # device s3 uses bf16-rounded wg (slt); keep swg consistent with it
    wg = wg.astype(ml_dtypes.bfloat16).astype(np.float32)
    swg = float(wg.sum())
    slt = np.zeros((G, NT, 128, 40), np.float32)
    for g in range(G):
        for t in range(NT):
            slt[g, t, :, t] = 1.0
            slt[g, t, :, 32 + t] = wg[g * 128:(g + 1) * 128]
    slt = slt.astype(ml_dtypes.bfloat16)
    s2l = np.zeros((NT, 128, NT), np.float32)
    for t in range(NT):
        s2l[t, :, t] = 1.0
    s2l = s2l.astype(ml_dtypes.bfloat16)
    epi = np.zeros((NT, 3), np.float32)
    epi[:, 0] = c0
    epi[:, 1] = swg / D
    epi[:, 2] = LN_EPS
    return xp, w8, bias_all, binit, slt, s2l, epi


def _in_maps(inputs):
    xp, w8, bias_all, binit, slt, s2l, epi = _host_prep(inputs)
    maps = []
    for bi in range(B):
        xt = xp[bi].T                            # [D, T] fp32
        # [kp, p, j, T] views
        xk = np.ascontiguousarray(
            xt.reshape(KP, 2, 128, T).transpose(0, 2, 1, 3))
        x8 = np.clip(xk, -240, 240).astype(ml_dtypes.float8_e4m3fn)
        xtb = xk.astype(ml_dtypes.bfloat16)
        maps.append({
            "x8": x8, "xtb": xtb, "w8": w8,
            "bias": bias_all, "binit": binit,
            "slt": slt, "s2l": s2l, "epi": epi,
        })
    return maps


def kernel(**inputs):
    nc = _get_nc()
    res = run_bass_kernel_spmd(nc, _in_maps(inputs), list(range(B)))
    out = np.stack([res.results[b]["out"].reshape(T, OUT) for b in range(B)])
    return out.astype(np.float32)


def kernel_traced(**inputs):
    """same as kernel() but returns (output, BassKernelResults) with timing"""
    nc = _get_nc()
    res = run_bass_kernel_spmd(nc, _in_maps(inputs), list(range(B)), trace=True)
    out = np.stack([res.results[b]["out"].reshape(T, OUT) for b in range(B)])
    return out.astype(np.float32), res

